# revision 48
# baseline (speedup 1.0000x reference)
"""Trainium2 Bass kernel for nn_MinimalAttention (GQA attention block).

Full-input contract: kernel(**inputs) takes the unsharded numpy inputs and
returns the full output. Internally shards across 8 NeuronCores:
  - data-parallel over batch (2) x tensor-parallel over heads (4 groups of
    8 q-heads / 2 kv-heads each), per the TP sharding hint.
  - each core computes a partial [2048, 2048] output (its heads' slice of
    attn_out @ Wo rows); host sums the 4 partials per batch.

Per-core kernel structure (all matmuls bf16, fp32 PSUM accumulation), built
as ONE software-pipelined pass so the PE never idles and ScalarE exp overlaps
matmuls from the start:
  prefix: K-proj (dedup'd; kv-head halves duplicated into kTd via SBUF-SBUF
          DMA), V-proj (vA/vB with ones column for the softmax denominator),
          Q-proj for seq block 0 / head-pair 0.
  16 iterations over (j seq-block, c head-pair): per key chunk kc:
          S^T pair (two 64-contraction row-tiled matmuls) -> ScalarE exp ->
          PV accumulation (lagged 2 chunks), with O-projection chains of the
          previous j-block and the next Q-projection chain interleaved as PE
          filler; normalize via GpSimd partition_broadcast + DVE
          reciprocal_approx_fast.
  tail: O-projection for the last j-block.
"""

import os
import sys

for _p in ("/opt/trn_rl_repo", "/opt/pypackages"):
    if _p not in sys.path and os.path.isdir(_p):
        sys.path.append(_p)

import numpy as np
import ml_dtypes

import concourse.bass as bass
import concourse.bacc as bacc
import concourse.mybir as mybir
import concourse.tile as tile
from concourse.bass_utils import run_bass_kernel_spmd

HIDDEN = 2048
SEQ = 2048
NUM_HEADS = 32
NUM_KV_HEADS = 8
HEAD_DIM = 64
N_CORES = 8
TP = 4                       # head-groups
BATCH = 2
QH = NUM_HEADS // TP         # 8 local q heads -> 4 pairs
KVH = NUM_KV_HEADS // TP     # 2 local kv heads
HC = HIDDEN // 128           # 16 hidden chunks
SC = SEQ // 128              # 16 seq chunks
NJ = SEQ // 512              # 4 seq 512-blocks

BF16 = mybir.dt.bfloat16
F32 = mybir.dt.float32
U16 = mybir.dt.uint16
EXP = mybir.ActivationFunctionType.Exp
SCALE = HEAD_DIM ** -0.5

# even global chunks compute exp on DVE via the Schraudolph bf16 bit trick
# (affine to u16 = bf16 bit pattern, one tensor_scalar op); odd chunks stay
# on ScalarE. This halves the per-chunk softmax pipeline latency so PSUM
# recycling stops pacing the S weight loads. rms err of the trick ~1.8%.
DVE_EXP = True
EXP_A = float(np.float32(128.0 / np.log(2.0) * SCALE))
EXP_B2 = float(np.float32(16256.0 - 128.0 * 0.0575))

# set by test.py to collect an NTFF profile; harness default = plain run
PROFILE = bool(os.environ.get("KERNEL_PROFILE"))
LAST_EXEC_NS = None
LAST_RESULTS = None


def _body(tc):
    nc = tc.nc
    # host-prepacked layouts: partition dim first, per-partition contiguous,
    # and grouped so every DMA reads/writes contiguous runs on both sides
    xt_d = nc.declare_dram_parameter("xtp", [NJ, 4, 128, 4, 512], BF16, isOutput=False)
    wq0_d = nc.declare_dram_parameter("wq0", [128, HC, 128], BF16, isOutput=False)
    wq1_d = nc.declare_dram_parameter("wq1", [128, HC, 384], BF16, isOutput=False)
    wk_d = nc.declare_dram_parameter("wkp", [4, 128, 4, 128], BF16, isOutput=False)
    wv_d = nc.declare_dram_parameter("wvp", [128, HC, 128], BF16, isOutput=False)
    wo_d = nc.declare_dram_parameter("wop", [128, 4, HIDDEN], BF16, isOutput=False)
    out = nc.declare_dram_parameter("out", [SEQ, HIDDEN], BF16, isOutput=True)

    consts = tc.alloc_tile_pool(name="consts", bufs=1)
    acts = tc.alloc_tile_pool(name="acts", bufs=1)

    wk_sb = consts.tile([128, HC, 128], BF16)
    wv_sb = consts.tile([128, HC, 128], BF16)
    wq_sb0 = consts.tile([128, HC, 128], BF16)
    wq_sb1 = consts.tile([128, HC, 384], BF16)
    wo_sb = consts.tile([128, 4, HIDDEN], BF16)

    # persistent activations
    qT = [acts.tile([128, SEQ], BF16, name=f"qT{c}") for c in range(4)]
    # kTd[v]: k^T of kv-head v duplicated on both partition halves (rows
    # 0:64 and 64:128) so the row-tiled S pair can read either half.
    kTd = [acts.tile([128, SEQ], BF16, name=f"kTd{v}") for v in range(KVH)]
    # vA: [v | ones | 0*63] -> PV psum rows 0:64 = out^T, row 64 = l
    # vB: [ones | 0*63 | v] -> PV psum row 0 = l, rows 64:128 = out^T
    vA = [acts.tile([128, SC, 128], BF16, name=f"vA{v}") for v in range(KVH)]
    vB = [acts.tile([128, SC, 128], BF16, name=f"vB{v}") for v in range(KVH)]
    outT = [acts.tile([128, SEQ], BF16, name=f"outT{c}") for c in range(4)]
    # x^T blocks stay resident the whole kernel, as quarter-tiles so
    # compute can start before a block's DMA fully lands
    xts4 = [[acts.tile([128, 4, 512], BF16, name=f"xt{j}_{q}") for q in range(4)]
            for j in range(NJ)]
    for v in range(KVH):
        nc.vector.memset(vA[v][:, :, 64:65], 1.0)
        nc.vector.memset(vA[v][:, :, 65:128], 0.0)
        nc.vector.memset(vB[v][:, :, 0:1], 1.0)
        nc.vector.memset(vB[v][:, :, 1:64], 0.0)
    # normalize staging: rl rows 0/64 hold the raw denominators; e_sb is the
    # 0/1 selector that PE-broadcasts them (psum_r = e_sb.T @ rl); rli holds
    # the broadcast reciprocals. bf16 suffices for the denominators.
    e_sb = acts.tile([128, 128], BF16, name="e_sb")
    rl = acts.tile([128, 512], BF16, name="rl")
    rli = acts.tile([128, 512], F32, name="rli")
    dum = acts.tile([128, 1], F32, name="dum")
    nc.vector.memset(e_sb, 0.0)
    nc.vector.memset(e_sb[64:65, 0:64], 1.0)
    nc.vector.memset(e_sb[0:1, 64:128], 1.0)
    nc.vector.memset(rl, 0.0)
    nc.vector.memset(dum, 0.0)

    def dma_xt(j):
        for q in range(4):
            nc.sync.dma_start(out=xts4[j][q], in_=xt_d[j, q])

    def xch(j, hc):
        return xts4[j][hc // 4], hc % 4

    ITERS = NJ * 4

    with tc.tile_pool(name="put", bufs=10) as put_pool, \
         tc.tile_pool(name="oraw", bufs=1) as oraw_pool, \
         tc.tile_pool(name="stage", bufs=2) as stage_pool, \
         tc.tile_pool(name="pss", bufs=2, space="PSUM") as pss, \
         tc.tile_pool(name="pso", bufs=1, space="PSUM") as pso, \
         tc.tile_pool(name="ppo", bufs=1, space="PSUM") as ppo, \
         tc.tile_pool(name="ppq", bufs=1, space="PSUM") as ppq:

        puL = [None] * (ITERS * SC)
        pso_t = [None] * ITERS
        oraw_t = [None] * ITERS
        psr_t = [None] * ITERS
        kh_state = {}
        qh_state = {}

        def k_half(j, half, pool, tag):
            if half == 0:
                kh_state[j] = pool.tile([128, 512], F32, tag=tag, name="ps_k")
            ps_k = kh_state[j]
            for hc in range(half * 8, half * 8 + 8):
                xt, ho = xch(j, hc)
                nc.tensor.matmul(
                    out=ps_k,
                    lhsT=wk_sb[:, hc, :],
                    rhs=xt[:, ho, :],
                    start=(hc == 0), stop=(hc == HC - 1),
                )
            if half == 1:
                del kh_state[j]
                js = slice(j * 512, (j + 1) * 512)
                nc.vector.tensor_copy(out=kTd[0][0:64, js], in_=ps_k[0:64, :])
                nc.vector.tensor_copy(out=kTd[1][64:128, js], in_=ps_k[64:128, :])
                nc.sync.dma_start(out=kTd[0][64:128, js], in_=kTd[0][0:64, js])
                nc.sync.dma_start(out=kTd[1][0:64, js], in_=kTd[1][64:128, js])

        def v_chunk(j, m, pool, tag):
            ps_v = pool.tile([128, 128], F32, tag=tag, name="ps_v")
            for hc in range(HC):
                xt, ho = xch(j, hc)
                nc.tensor.matmul(
                    out=ps_v,
                    lhsT=xt[:, ho, m * 128:(m + 1) * 128],
                    rhs=wv_sb[:, hc, :],
                    start=(hc == 0), stop=(hc == HC - 1),
                )
            kcg = j * 4 + m
            for v in range(KVH):
                vs = slice(v * 64, (v + 1) * 64)
                nc.vector.tensor_copy(out=vA[v][:, kcg, 0:64], in_=ps_v[:, vs])
                nc.vector.tensor_copy(out=vB[v][:, kcg, 64:128], in_=ps_v[:, vs])

        def q_half(qi, half, pool, tag):
            jq, cq = divmod(qi, 4)
            if half == 0:
                qh_state[qi] = pool.tile([128, 512], F32, tag=tag, name="ps_q")
            ps_q = qh_state[qi]
            for hc in range(half * 8, half * 8 + 8):
                xt, ho = xch(jq, hc)
                wq_l = (wq_sb0[:, hc, :] if cq == 0
                        else wq_sb1[:, hc, (cq - 1) * 128:cq * 128])
                nc.tensor.matmul(
                    out=ps_q,
                    lhsT=wq_l,
                    rhs=xt[:, ho, :],
                    start=(hc == 0), stop=(hc == HC - 1),
                )
            if half == 1:
                del qh_state[qi]
                nc.scalar.copy(
                    out=qT[cq][:, jq * 512:(jq + 1) * 512], in_=ps_q
                )

        def emit_S_half(it, kc, half):
            j, c = divmod(it, 4)
            kv = c // 2
            js = slice(j * 512, (j + 1) * 512)
            ks = slice(kc * 128, (kc + 1) * 128)
            n = it * SC + kc
            if half == 0:
                pu = put_pool.tile([128, 2, 512], BF16, tag="pu", name="pu")
                puL[n] = pu
                ps_s = pss.tile([128, 1024], F32, tag="ps_s")
                sps_state[n] = ps_s
                nc.tensor.matmul(
                    out=ps_s[:, 0:512],
                    lhsT=kTd[kv][0:64, ks],
                    rhs=qT[c][0:64, js],
                    start=True, stop=True,
                )
            else:
                ps_s = sps_state.pop(n)
                nc.tensor.matmul(
                    out=ps_s[:, 512:1024],
                    lhsT=kTd[kv][64:128, ks],
                    rhs=qT[c][64:128, js],
                    start=True, stop=True,
                )
                pu_flat = puL[n].rearrange("p a b -> p (a b)")
                if n >= ITERS * SC - 4:
                    # endgame: split each chunk across BOTH engines so the
                    # tail queues drain in half the time (PE idles otherwise)
                    nc.scalar.activation(
                        out=puL[n][:, 0, :], in_=ps_s[:, 0:512],
                        func=EXP, scale=SCALE,
                    )
                    nc.vector.tensor_scalar(
                        puL[n][:, 1, :].bitcast(U16), ps_s[:, 512:1024],
                        EXP_A, EXP_B2,
                        mybir.AluOpType.mult, mybir.AluOpType.add,
                    )
                elif DVE_EXP and n % 2 == 1:
                    nc.vector.tensor_scalar(
                        pu_flat.bitcast(U16), ps_s, EXP_A, EXP_B2,
                        mybir.AluOpType.mult, mybir.AluOpType.add,
                    )
                else:
                    nc.scalar.activation(
                        out=pu_flat, in_=ps_s, func=EXP, scale=SCALE
                    )

        def norm_head(it):
            ps_oA, ps_oB = pso_t[it]
            oa = oraw_pool.tile([128, 512], BF16, tag="ra")
            ob = oraw_pool.tile([128, 512], BF16, tag="rb")
            # ScalarE copies keep the norm burst off DVE (which carries the
            # even-chunk exps); but for the final iteration ScalarE is busy
            # with the tail exps while DVE is idle, so swap engines there
            cp = nc.vector.tensor_copy if it == ITERS - 1 else nc.scalar.copy
            cp(out=oa, in_=ps_oA)
            cp(out=ob, in_=ps_oB)
            oraw_t[it] = (oa, ob)
            nc.vector.tensor_copy(out=rl[64:65, :], in_=oa[64:65, :])
            nc.vector.tensor_copy(out=rl[0:1, :], in_=ob[0:1, :])
            ps_r = ppo.tile([128, 512], F32, tag="ppo")
            nc.tensor.matmul(out=ps_r, lhsT=e_sb, rhs=rl, start=True, stop=True)
            psr_t[it] = ps_r

        # normalize split across three filler slots so the DVE burst never
        # delays the even-chunk exps (which gate the S psum recycle)
        def norm_recip(it):
            nc.vector.reciprocal_approx_fast(out=rli, in_=psr_t[it])

        def norm_mul(it, half):
            j, c = divmod(it, 4)
            js = slice(j * 512, (j + 1) * 512)
            oa, ob = oraw_t[it]
            if half == 0:
                nc.vector.tensor_mul(outT[c][0:64, js], oa[0:64], rli[0:64])
            else:
                nc.vector.tensor_mul(outT[c][64:128, js], ob[64:128], rli[64:128])

        def emit_PV_half(it, kc, half):
            j, c = divmod(it, 4)
            kv = c // 2
            if half == 0:
                if kc == 0:
                    ps_oA = pso.tile([128, 512], F32, tag="oA")
                    ps_oB = pso.tile([128, 512], F32, tag="oB")
                    pso_t[it] = (ps_oA, ps_oB)
                nc.tensor.matmul(
                    out=pso_t[it][0],
                    lhsT=vA[kv][:, kc, :],
                    rhs=puL[it * SC + kc][:, 0, :],
                    start=(kc == 0), stop=(kc == SC - 1),
                )
            else:
                nc.tensor.matmul(
                    out=pso_t[it][1],
                    lhsT=vB[kv][:, kc, :],
                    rhs=puL[it * SC + kc][:, 1, :],
                    start=(kc == 0), stop=(kc == SC - 1),
                )
                if kc == SC - 1:
                    norm_head(it)

        stm_state = {}

        def oproj_chain(jb, m, n, pool=None, tag="ppo"):
            ms = slice((jb * 4 + m) * 128, (jb * 4 + m + 1) * 128)
            ns = slice(n * 512, (n + 1) * 512)
            ps_p = (pool or ppo).tile([128, 512], F32, tag=tag)
            for cc in range(4):
                nc.tensor.matmul(
                    out=ps_p,
                    lhsT=outT[cc][:, ms],
                    rhs=wo_sb[:, cc, ns],
                    start=(cc == 0), stop=(cc == 3),
                )
            # stage the whole 128-row block and emit ONE fully-contiguous
            # 512KB DMA after its 4 chunks: the out ring processes full-row
            # transfers far more efficiently than 4 scattered 1KB-run DMAs
            key = (jb, m)
            if key not in stm_state:
                stm_state[key] = (stage_pool.tile([128, HIDDEN], BF16,
                                                  tag="stm", name="stm"), [])
            st_m, done = stm_state[key]
            # alternate staging copies between DVE and ScalarE
            if (m + n) % 2 == 0:
                nc.vector.tensor_copy(out=st_m[:, ns], in_=ps_p)
            else:
                nc.scalar.copy(out=st_m[:, ns], in_=ps_p)
            done.append(n)
            if len(done) == 4:
                del stm_state[key]
                eng = nc.sync if m % 2 == 0 else nc.scalar
                eng.dma_start(out=out[ms, :], in_=st_m)

        # ---- prefix: DMAs in need order; K(0), Q(0,0), V(0) ----
        # wk split across queues; j=0 x^T split per-hc so k_half(0,0) can
        # start after ~one hc chunk lands instead of a full quarter
        # issue order = first-use order: k(0,0) needs wk[0:2]+xt0[0:8];
        # k(0,1) the rest of wk/xt0; round-0 fillers need xt1 early
        nc.sync.dma_start(out=wk_sb[:, 0:4, :], in_=wk_d[0])
        nc.sync.dma_start(out=wk_sb[:, 4:8, :], in_=wk_d[1])
        for q in range(2):
            for h in range(4):
                nc.sync.dma_start(
                    out=xts4[0][q][:, h, :], in_=xt_d[0, q, :, h, :]
                )
        nc.sync.dma_start(out=wk_sb[:, 8:12, :], in_=wk_d[2])
        nc.sync.dma_start(out=wk_sb[:, 12:16, :], in_=wk_d[3])
        for q in range(2, 4):
            for h in range(4):
                nc.sync.dma_start(
                    out=xts4[0][q][:, h, :], in_=xt_d[0, q, :, h, :]
                )
        nc.sync.dma_start(out=wq_sb0, in_=wq0_d[:, :, :])
        nc.sync.dma_start(out=xts4[1][0], in_=xt_d[1, 0])
        nc.sync.dma_start(out=xts4[1][1], in_=xt_d[1, 1])
        nc.sync.dma_start(out=wv_sb, in_=wv_d[:, :, :])
        nc.sync.dma_start(out=xts4[1][2], in_=xt_d[1, 2])
        nc.sync.dma_start(out=xts4[1][3], in_=xt_d[1, 3])
        dma_xt(2)
        nc.sync.dma_start(out=wq_sb1, in_=wq1_d[:, :, :])
        dma_xt(3)
        nc.sync.dma_start(out=wo_sb, in_=wo_d[:, :, :])
        k_half(0, 0, ppo, "ppo")
        k_half(0, 1, ppo, "ppo")
        q_half(0, 0, ppq, "ppq")
        q_half(0, 1, ppq, "ppq")
        v_chunk(0, 0, ppo, "ppo")
        v_chunk(0, 1, ppq, "ppq")
        v_chunk(0, 2, ppo, "ppo")
        v_chunk(0, 3, ppq, "ppq")

        # ---- filler schedule ----
        # round 0: remaining K/V projections just-in-time; later rounds:
        # O-proj chains of block j-1 plus the next Q chain
        sched = {}

        def add(it, kc, fn):
            sched.setdefault((it, kc), []).append(fn)

        add(0, 0, lambda: k_half(1, 0, ppo, "ppo"))
        add(0, 1, lambda: k_half(1, 1, ppo, "ppo"))
        add(0, 2, lambda: v_chunk(1, 0, ppq, "ppq"))
        add(0, 3, lambda: v_chunk(1, 1, ppq, "ppq"))
        add(0, 4, lambda: k_half(2, 0, ppo, "ppo"))
        add(0, 5, lambda: k_half(2, 1, ppo, "ppo"))
        add(0, 6, lambda: v_chunk(1, 2, ppq, "ppq"))
        add(0, 7, lambda: v_chunk(1, 3, ppq, "ppq"))
        add(0, 8, lambda: k_half(3, 0, ppo, "ppo"))
        add(0, 9, lambda: k_half(3, 1, ppo, "ppo"))
        add(0, 10, lambda: v_chunk(2, 0, ppq, "ppq"))
        add(0, 10, lambda: v_chunk(2, 1, ppq, "ppq"))
        add(0, 11, lambda: v_chunk(2, 2, ppq, "ppq"))
        add(0, 11, lambda: v_chunk(2, 3, ppq, "ppq"))
        add(0, 12, lambda: q_half(1, 0, ppo, "ppo"))
        add(0, 13, lambda: q_half(1, 1, ppo, "ppo"))
        add(0, 14, lambda: v_chunk(3, 0, ppq, "ppq"))
        add(0, 14, lambda: v_chunk(3, 1, ppq, "ppq"))
        add(0, 15, lambda: v_chunk(3, 2, ppq, "ppq"))
        add(0, 15, lambda: v_chunk(3, 3, ppq, "ppq"))
        def pe_fill(k):
            for _ in range(k):
                ps_j = ppq.tile([128, 512], F32, tag="ppq", name="junk")
                nc.tensor.matmul(out=ps_j, lhsT=e_sb, rhs=rl,
                                 start=True, stop=True)

        for it in range(1, ITERS):
            qi = it + 1
            if qi < ITERS:
                add(it, 5, lambda qi=qi: q_half(qi, 0, ppq, "ppq"))
                add(it, 6, lambda qi=qi: q_half(qi, 1, ppq, "ppq"))

        for it in range(1, ITERS):
            j, c = divmod(it, 4)
            if j > 0:
                slots = {10: 1, 12: 2, 14: 3, (15 if c == 0 else 1): 0}
                for kc, n in slots.items():
                    add(it, kc, lambda j=j, c=c, n=n: oproj_chain(j - 1, c, n))
            add(it, 4, lambda it=it: norm_recip(it - 1))
            add(it, 5, lambda it=it: norm_mul(it - 1, 0))
            add(it, 6, lambda it=it: norm_mul(it - 1, 1))

        # ---- flat global-slot pipeline, superslots of 2 chunks ----
        # S pairs for two consecutive chunks are emitted back-to-back so the
        # second pair's LDWEIGHTS hides under the first pair's streams, and
        # only one full matmul per superslot pays the post-tiled-pair LD tax
        sps_state = {}
        for m in range(ITERS * SC // 2):
            n0, n1 = 2 * m, 2 * m + 1
            it0, kc0 = divmod(n0, SC)
            it1, kc1 = divmod(n1, SC)
            emit_S_half(it0, kc0, 0)
            emit_S_half(it0, kc0, 1)
            emit_S_half(it1, kc1, 0)
            emit_S_half(it1, kc1, 1)
            if n0 >= 4:
                emit_PV_half(*divmod(n0 - 4, SC), 0)
                emit_PV_half(*divmod(n0 - 4, SC), 1)
            for fn in sched.get((it0, kc0), ()):
                fn()
            if n1 >= 4:
                emit_PV_half(*divmod(n1 - 4, SC), 0)
                emit_PV_half(*divmod(n1 - 4, SC), 1)
            for fn in sched.get((it1, kc1), ()):
                fn()
        # keep the PE (and HAM) busy while the tail exps drain on ScalarE;
        # junk matmuls have no deps so they run during the exp waits —
        # interleaved between tail PV pairs to bridge each exp's latency
        for n in range(ITERS * SC - 4, ITERS * SC):
            if n >= ITERS * SC - 2:
                pe_fill(3)
            emit_PV_half(*divmod(n, SC), 0)
            emit_PV_half(*divmod(n, SC), 1)
        itL = ITERS - 1
        jL, cL = divmod(itL, 4)
        ps_oA, ps_oB = pso_t[itL]
        nc.vector.reciprocal_approx_fast(out=rli, in_=psr_t[itL])
        # chunk the final normalize per 128-seq block so the drain's first
        # m-chain can start ~1.5us earlier
        for mm in range(4):
            s = slice(jL * 512 + mm * 128, jL * 512 + (mm + 1) * 128)
            r = slice(mm * 128, (mm + 1) * 128)
            nc.vector.tensor_mul(outT[cL][0:64, s], ps_oA[0:64, r], rli[0:64, r])
            nc.vector.tensor_mul(outT[cL][64:128, s], ps_oB[64:128, r], rli[64:128, r])

    # drain: O-projection of the last j-block via wide psum tiles; the
    # iteration pools above are closed, freeing PSUM and SBUF for wide
    # staging; copies split across DVE and ScalarE
    with tc.tile_pool(name="dps", bufs=2, space="PSUM") as dps, \
         tc.tile_pool(name="dst", bufs=3) as dst:
        for m in range(4):
            ms = slice(((NJ - 1) * 4 + m) * 128, ((NJ - 1) * 4 + m + 1) * 128)
            ps_w = dps.tile([128, HIDDEN], F32, tag="dw")
            st = dst.tile([128, HIDDEN], BF16, tag="da")
            for n in range(4):
                ns = slice(n * 512, (n + 1) * 512)
                for cc in range(4):
                    nc.tensor.matmul(
                        out=ps_w[:, ns],
                        lhsT=outT[cc][:, ms],
                        rhs=wo_sb[:, cc, ns],
                        start=(cc == 0), stop=(cc == 3),
                    )
                if n % 2 == 0:
                    nc.vector.tensor_copy(out=st[:, ns], in_=ps_w[:, ns])
                    nc.scalar.dma_start(out=out[ms, ns], in_=st[:, ns])
                else:
                    nc.scalar.copy(out=st[:, ns], in_=ps_w[:, ns])
                    nc.sync.dma_start(out=out[ms, ns], in_=st[:, ns])

    acts.release()
    consts.release()


_NC_CACHE = None


def _build():
    global _NC_CACHE
    if _NC_CACHE is None:
        nc = bacc.Bacc(
            "TRN2",
            target_bir_lowering=False,
            debug=False,
            enable_asserts=False,
            num_devices=N_CORES,
        )
        with tile.TileContext(nc) as tc:
            _body(tc)
        nc.compile()
        _NC_CACHE = nc
    return _NC_CACHE


def kernel(x, Wq, Wk, Wv, Wo):
    global LAST_EXEC_NS, LAST_RESULTS
    x = np.asarray(x, dtype=np.float32)
    Wq = np.asarray(Wq, dtype=np.float32)
    Wk = np.asarray(Wk, dtype=np.float32)
    Wv = np.asarray(Wv, dtype=np.float32)
    Wo = np.asarray(Wo, dtype=np.float32)
    bf = ml_dtypes.bfloat16

    in_maps = []
    for core in range(N_CORES):
        b, g = divmod(core, TP)
        qs = slice(g * QH * HEAD_DIM, (g + 1) * QH * HEAD_DIM)
        kvs = slice(g * KVH * HEAD_DIM, (g + 1) * KVH * HEAD_DIM)
        # (j, q, p, h, s): x[b][j*512+s, (q*4+h)*128+p] — quarter-major so
        # each quarter DMA is one contiguous 512KB DRAM run
        xtp = np.ascontiguousarray(
            x[b].reshape(NJ, 512, 4, 4, 128).transpose(0, 2, 4, 3, 1)
        ).astype(bf)
        wqp = np.ascontiguousarray(
            Wq[:, qs].reshape(HC, 128, 512).transpose(1, 0, 2)
        ).astype(bf)
        wq0 = np.ascontiguousarray(wqp[:, :, 0:128])
        wq1 = np.ascontiguousarray(wqp[:, :, 128:512])
        # (c, p, h, m): chunk-major so each of the 4 wk DMAs is contiguous
        wkp = np.ascontiguousarray(
            Wk[:, kvs].reshape(4, 4, 128, 128).transpose(0, 2, 1, 3)
        ).astype(bf)
        wvp = np.ascontiguousarray(
            Wv[:, kvs].reshape(HC, 128, 128).transpose(1, 0, 2)
        ).astype(bf)
        wop = np.ascontiguousarray(
            Wo[qs, :].reshape(4, 128, HIDDEN).transpose(1, 0, 2)
        ).astype(bf)
        in_maps.append({
            "xtp": xtp, "wq0": wq0, "wq1": wq1, "wkp": wkp, "wvp": wvp,
            "wop": wop,
        })

    nc = _build()
    res = run_bass_kernel_spmd(
        nc,
        in_maps,
        core_ids=list(range(N_CORES)),
        trace=PROFILE,
        trace_cores=list(range(N_CORES)) if PROFILE else None,
    )
    LAST_EXEC_NS = res.exec_time_ns
    LAST_RESULTS = res
    partials = [np.asarray(r["out"], dtype=np.float32) for r in res.results]
    out = np.empty((BATCH, SEQ, HIDDEN), dtype=np.float32)
    for b in range(BATCH):
        out[b] = partials[TP * b]
        for g in range(1, TP):
            out[b] += partials[TP * b + g]
    return out



# revision 49
# speedup vs baseline: 1.0239x; 1.0239x over previous
"""Trainium2 Bass kernel for nn_MinimalAttention (GQA attention block).

Full-input contract: kernel(**inputs) takes the unsharded numpy inputs and
returns the full output. Internally shards across 8 NeuronCores:
  - data-parallel over batch (2) x tensor-parallel over heads (4 groups of
    8 q-heads / 2 kv-heads each), per the TP sharding hint.
  - each core computes a partial [2048, 2048] output (its heads' slice of
    attn_out @ Wo rows); host sums the 4 partials per batch.

Per-core kernel structure (all matmuls bf16, fp32 PSUM accumulation), built
as ONE software-pipelined pass so the PE never idles and ScalarE exp overlaps
matmuls from the start:
  prefix: K-proj (dedup'd; kv-head halves duplicated into kTd via SBUF-SBUF
          DMA), V-proj (vA/vB with ones column for the softmax denominator),
          Q-proj for seq block 0 / head-pair 0.
  16 iterations over (j seq-block, c head-pair): per key chunk kc:
          S^T pair (two 64-contraction row-tiled matmuls) -> ScalarE exp ->
          PV accumulation (lagged 2 chunks), with O-projection chains of the
          previous j-block and the next Q-projection chain interleaved as PE
          filler; normalize via GpSimd partition_broadcast + DVE
          reciprocal_approx_fast.
  tail: O-projection for the last j-block.
"""

import os
import sys

for _p in ("/opt/trn_rl_repo", "/opt/pypackages"):
    if _p not in sys.path and os.path.isdir(_p):
        sys.path.append(_p)

import numpy as np
import ml_dtypes

import concourse.bass as bass
import concourse.bacc as bacc
import concourse.mybir as mybir
import concourse.tile as tile
from concourse.bass_utils import run_bass_kernel_spmd

HIDDEN = 2048
SEQ = 2048
NUM_HEADS = 32
NUM_KV_HEADS = 8
HEAD_DIM = 64
N_CORES = 8
TP = 4                       # head-groups
BATCH = 2
QH = NUM_HEADS // TP         # 8 local q heads -> 4 pairs
KVH = NUM_KV_HEADS // TP     # 2 local kv heads
HC = HIDDEN // 128           # 16 hidden chunks
SC = SEQ // 128              # 16 seq chunks
NJ = SEQ // 512              # 4 seq 512-blocks

BF16 = mybir.dt.bfloat16
F32 = mybir.dt.float32
U16 = mybir.dt.uint16
EXP = mybir.ActivationFunctionType.Exp
SCALE = HEAD_DIM ** -0.5

# even global chunks compute exp on DVE via the Schraudolph bf16 bit trick
# (affine to u16 = bf16 bit pattern, one tensor_scalar op); odd chunks stay
# on ScalarE. This halves the per-chunk softmax pipeline latency so PSUM
# recycling stops pacing the S weight loads. rms err of the trick ~1.8%.
DVE_EXP = True
EXP_A = float(np.float32(128.0 / np.log(2.0) * SCALE))
EXP_B2 = float(np.float32(16256.0 - 128.0 * 0.0575))

# set by test.py to collect an NTFF profile; harness default = plain run
PROFILE = bool(os.environ.get("KERNEL_PROFILE"))
LAST_EXEC_NS = None
LAST_RESULTS = None


def _body(tc):
    nc = tc.nc
    # host-prepacked layouts: partition dim first, per-partition contiguous,
    # and grouped so every DMA reads/writes contiguous runs on both sides
    xt_d = nc.declare_dram_parameter("xtp", [NJ, 4, 128, 4, 512], BF16, isOutput=False)
    wq0_d = nc.declare_dram_parameter("wq0", [128, HC, 128], BF16, isOutput=False)
    wq1_d = nc.declare_dram_parameter("wq1", [128, HC, 384], BF16, isOutput=False)
    wk_d = nc.declare_dram_parameter("wkp", [4, 128, 4, 128], BF16, isOutput=False)
    wv_d = nc.declare_dram_parameter("wvp", [128, HC, 128], BF16, isOutput=False)
    wo_d = nc.declare_dram_parameter("wop", [128, 4, HIDDEN], BF16, isOutput=False)
    out = nc.declare_dram_parameter("out", [SEQ, HIDDEN], BF16, isOutput=True)

    consts = tc.alloc_tile_pool(name="consts", bufs=1)
    acts = tc.alloc_tile_pool(name="acts", bufs=1)

    wk_sb = consts.tile([128, HC, 128], BF16)
    wv_sb = consts.tile([128, HC, 128], BF16)
    wq_sb0 = consts.tile([128, HC, 128], BF16)
    wq_sb1 = consts.tile([128, HC, 384], BF16)
    wo_sb = consts.tile([128, 4, HIDDEN], BF16)

    # persistent activations
    qT = [acts.tile([128, SEQ], BF16, name=f"qT{c}") for c in range(4)]
    # kTd[v]: k^T of kv-head v duplicated on both partition halves (rows
    # 0:64 and 64:128) so the row-tiled S pair can read either half.
    kTd = [acts.tile([128, SEQ], BF16, name=f"kTd{v}") for v in range(KVH)]
    # vA: [v | ones | 0*63] -> PV psum rows 0:64 = out^T, row 64 = l
    # vB: [ones | 0*63 | v] -> PV psum row 0 = l, rows 64:128 = out^T
    vA = [acts.tile([128, SC, 128], BF16, name=f"vA{v}") for v in range(KVH)]
    vB = [acts.tile([128, SC, 128], BF16, name=f"vB{v}") for v in range(KVH)]
    outT = [acts.tile([128, SEQ], BF16, name=f"outT{c}") for c in range(4)]
    # x^T blocks stay resident the whole kernel, as quarter-tiles so
    # compute can start before a block's DMA fully lands
    xts4 = [[acts.tile([128, 4, 512], BF16, name=f"xt{j}_{q}") for q in range(4)]
            for j in range(NJ)]
    for v in range(KVH):
        nc.vector.memset(vA[v][:, :, 64:65], 1.0)
        nc.vector.memset(vA[v][:, :, 65:128], 0.0)
        nc.vector.memset(vB[v][:, :, 0:1], 1.0)
        nc.vector.memset(vB[v][:, :, 1:64], 0.0)
    # normalize staging: rl rows 0/64 hold the raw denominators; e_sb is the
    # 0/1 selector that PE-broadcasts them (psum_r = e_sb.T @ rl); rli holds
    # the broadcast reciprocals. bf16 suffices for the denominators.
    e_sb = acts.tile([128, 128], BF16, name="e_sb")
    rl = acts.tile([128, 512], BF16, name="rl")
    rli = acts.tile([128, 512], F32, name="rli")
    dum = acts.tile([128, 1], F32, name="dum")
    nc.vector.memset(e_sb, 0.0)
    nc.vector.memset(e_sb[64:65, 0:64], 1.0)
    nc.vector.memset(e_sb[0:1, 64:128], 1.0)
    nc.vector.memset(rl, 0.0)
    nc.vector.memset(dum, 0.0)

    def dma_xt(j):
        for q in range(4):
            nc.sync.dma_start(out=xts4[j][q], in_=xt_d[j, q])

    def xch(j, hc):
        return xts4[j][hc // 4], hc % 4

    ITERS = NJ * 4

    with tc.tile_pool(name="put", bufs=10) as put_pool, \
         tc.tile_pool(name="oraw", bufs=1) as oraw_pool, \
         tc.tile_pool(name="stage", bufs=2) as stage_pool, \
         tc.tile_pool(name="pss", bufs=2, space="PSUM") as pss, \
         tc.tile_pool(name="pso", bufs=1, space="PSUM") as pso, \
         tc.tile_pool(name="ppo", bufs=1, space="PSUM") as ppo, \
         tc.tile_pool(name="ppq", bufs=1, space="PSUM") as ppq:

        puL = [None] * (ITERS * SC)
        pso_t = [None] * ITERS
        oraw_t = [None] * ITERS
        psr_t = [None] * ITERS
        kh_state = {}
        qh_state = {}

        def k_half(j, half, pool, tag):
            if half == 0:
                kh_state[j] = pool.tile([128, 512], F32, tag=tag, name="ps_k")
            ps_k = kh_state[j]
            for hc in range(half * 8, half * 8 + 8):
                xt, ho = xch(j, hc)
                nc.tensor.matmul(
                    out=ps_k,
                    lhsT=wk_sb[:, hc, :],
                    rhs=xt[:, ho, :],
                    start=(hc == 0), stop=(hc == HC - 1),
                )
            if half == 1:
                del kh_state[j]
                js = slice(j * 512, (j + 1) * 512)
                nc.vector.tensor_copy(out=kTd[0][0:64, js], in_=ps_k[0:64, :])
                nc.vector.tensor_copy(out=kTd[1][64:128, js], in_=ps_k[64:128, :])
                nc.sync.dma_start(out=kTd[0][64:128, js], in_=kTd[0][0:64, js])
                nc.sync.dma_start(out=kTd[1][0:64, js], in_=kTd[1][64:128, js])

        def v_chunk(j, m, pool, tag):
            ps_v = pool.tile([128, 128], F32, tag=tag, name="ps_v")
            for hc in range(HC):
                xt, ho = xch(j, hc)
                nc.tensor.matmul(
                    out=ps_v,
                    lhsT=xt[:, ho, m * 128:(m + 1) * 128],
                    rhs=wv_sb[:, hc, :],
                    start=(hc == 0), stop=(hc == HC - 1),
                )
            kcg = j * 4 + m
            for v in range(KVH):
                vs = slice(v * 64, (v + 1) * 64)
                nc.vector.tensor_copy(out=vA[v][:, kcg, 0:64], in_=ps_v[:, vs])
                nc.vector.tensor_copy(out=vB[v][:, kcg, 64:128], in_=ps_v[:, vs])

        def q_half(qi, half, pool, tag):
            jq, cq = divmod(qi, 4)
            if half == 0:
                qh_state[qi] = pool.tile([128, 512], F32, tag=tag, name="ps_q")
            ps_q = qh_state[qi]
            for hc in range(half * 8, half * 8 + 8):
                xt, ho = xch(jq, hc)
                wq_l = (wq_sb0[:, hc, :] if cq == 0
                        else wq_sb1[:, hc, (cq - 1) * 128:cq * 128])
                nc.tensor.matmul(
                    out=ps_q,
                    lhsT=wq_l,
                    rhs=xt[:, ho, :],
                    start=(hc == 0), stop=(hc == HC - 1),
                )
            if half == 1:
                del qh_state[qi]
                nc.scalar.copy(
                    out=qT[cq][:, jq * 512:(jq + 1) * 512], in_=ps_q
                )

        def emit_S_half(it, kc, half):
            j, c = divmod(it, 4)
            kv = c // 2
            js = slice(j * 512, (j + 1) * 512)
            ks = slice(kc * 128, (kc + 1) * 128)
            n = it * SC + kc
            if half == 0:
                pu = put_pool.tile([128, 2, 512], BF16, tag="pu", name="pu")
                puL[n] = pu
                ps_s = pss.tile([128, 1024], F32, tag="ps_s")
                sps_state[n] = ps_s
                nc.tensor.matmul(
                    out=ps_s[:, 0:512],
                    lhsT=kTd[kv][0:64, ks],
                    rhs=qT[c][0:64, js],
                    start=True, stop=True,
                )
            else:
                ps_s = sps_state.pop(n)
                nc.tensor.matmul(
                    out=ps_s[:, 512:1024],
                    lhsT=kTd[kv][64:128, ks],
                    rhs=qT[c][64:128, js],
                    start=True, stop=True,
                )
                pu_flat = puL[n].rearrange("p a b -> p (a b)")
                if n >= ITERS * SC - 4:
                    # endgame: split each chunk across BOTH engines so the
                    # tail queues drain in half the time (PE idles otherwise)
                    nc.scalar.activation(
                        out=puL[n][:, 0, :], in_=ps_s[:, 0:512],
                        func=EXP, scale=SCALE,
                    )
                    nc.vector.tensor_scalar(
                        puL[n][:, 1, :].bitcast(U16), ps_s[:, 512:1024],
                        EXP_A, EXP_B2,
                        mybir.AluOpType.mult, mybir.AluOpType.add,
                    )
                elif DVE_EXP and n % 2 == 0:
                    nc.vector.tensor_scalar(
                        pu_flat.bitcast(U16), ps_s, EXP_A, EXP_B2,
                        mybir.AluOpType.mult, mybir.AluOpType.add,
                    )
                else:
                    nc.scalar.activation(
                        out=pu_flat, in_=ps_s, func=EXP, scale=SCALE
                    )

        def norm_head(it):
            ps_oA, ps_oB = pso_t[it]
            oa = oraw_pool.tile([128, 512], BF16, tag="ra")
            ob = oraw_pool.tile([128, 512], BF16, tag="rb")
            # ScalarE copies keep the norm burst off DVE (which carries the
            # even-chunk exps); but for the final iteration ScalarE is busy
            # with the tail exps while DVE is idle, so swap engines there
            cp = nc.vector.tensor_copy if it == ITERS - 1 else nc.scalar.copy
            cp(out=oa, in_=ps_oA)
            cp(out=ob, in_=ps_oB)
            oraw_t[it] = (oa, ob)
            nc.vector.tensor_copy(out=rl[64:65, :], in_=oa[64:65, :])
            nc.vector.tensor_copy(out=rl[0:1, :], in_=ob[0:1, :])
            ps_r = ppo.tile([128, 512], F32, tag="ppo")
            nc.tensor.matmul(out=ps_r, lhsT=e_sb, rhs=rl, start=True, stop=True)
            psr_t[it] = ps_r

        # normalize split across three filler slots so the DVE burst never
        # delays the even-chunk exps (which gate the S psum recycle)
        def norm_recip(it):
            nc.vector.reciprocal_approx_fast(out=rli, in_=psr_t[it])

        def norm_mul(it, half):
            j, c = divmod(it, 4)
            js = slice(j * 512, (j + 1) * 512)
            oa, ob = oraw_t[it]
            if half == 0:
                nc.vector.tensor_mul(outT[c][0:64, js], oa[0:64], rli[0:64])
            else:
                nc.vector.tensor_mul(outT[c][64:128, js], ob[64:128], rli[64:128])

        def emit_PV_half(it, kc, half):
            j, c = divmod(it, 4)
            kv = c // 2
            if half == 0:
                if kc == 0:
                    ps_oA = pso.tile([128, 512], F32, tag="oA")
                    ps_oB = pso.tile([128, 512], F32, tag="oB")
                    pso_t[it] = (ps_oA, ps_oB)
                nc.tensor.matmul(
                    out=pso_t[it][0],
                    lhsT=vA[kv][:, kc, :],
                    rhs=puL[it * SC + kc][:, 0, :],
                    start=(kc == 0), stop=(kc == SC - 1),
                )
            else:
                nc.tensor.matmul(
                    out=pso_t[it][1],
                    lhsT=vB[kv][:, kc, :],
                    rhs=puL[it * SC + kc][:, 1, :],
                    start=(kc == 0), stop=(kc == SC - 1),
                )
                if kc == SC - 1:
                    norm_head(it)

        stm_state = {}

        def oproj_chain(jb, m, n, pool=None, tag="ppo"):
            ms = slice((jb * 4 + m) * 128, (jb * 4 + m + 1) * 128)
            ns = slice(n * 512, (n + 1) * 512)
            ps_p = (pool or ppo).tile([128, 512], F32, tag=tag)
            for cc in range(4):
                nc.tensor.matmul(
                    out=ps_p,
                    lhsT=outT[cc][:, ms],
                    rhs=wo_sb[:, cc, ns],
                    start=(cc == 0), stop=(cc == 3),
                )
            # stage the whole 128-row block and emit ONE fully-contiguous
            # 512KB DMA after its 4 chunks: the out ring processes full-row
            # transfers far more efficiently than 4 scattered 1KB-run DMAs
            key = (jb, m)
            if key not in stm_state:
                stm_state[key] = (stage_pool.tile([128, HIDDEN], BF16,
                                                  tag="stm", name="stm"), [])
            st_m, done = stm_state[key]
            # alternate staging copies between DVE and ScalarE
            if (m + n) % 2 == 0:
                nc.vector.tensor_copy(out=st_m[:, ns], in_=ps_p)
            else:
                nc.scalar.copy(out=st_m[:, ns], in_=ps_p)
            done.append(n)
            if len(done) == 4:
                del stm_state[key]
                eng = nc.sync if m % 2 == 0 else nc.scalar
                eng.dma_start(out=out[ms, :], in_=st_m)

        # ---- prefix: DMAs in need order; K(0), Q(0,0), V(0) ----
        # wk split across queues; j=0 x^T split per-hc so k_half(0,0) can
        # start after ~one hc chunk lands instead of a full quarter
        # issue order = first-use order: k(0,0) needs wk[0:2]+xt0[0:8];
        # k(0,1) the rest of wk/xt0; round-0 fillers need xt1 early
        nc.sync.dma_start(out=wk_sb[:, 0:4, :], in_=wk_d[0])
        nc.sync.dma_start(out=wk_sb[:, 4:8, :], in_=wk_d[1])
        for q in range(2):
            for h in range(4):
                nc.sync.dma_start(
                    out=xts4[0][q][:, h, :], in_=xt_d[0, q, :, h, :]
                )
        nc.sync.dma_start(out=wk_sb[:, 8:12, :], in_=wk_d[2])
        nc.sync.dma_start(out=wk_sb[:, 12:16, :], in_=wk_d[3])
        for q in range(2, 4):
            for h in range(4):
                nc.sync.dma_start(
                    out=xts4[0][q][:, h, :], in_=xt_d[0, q, :, h, :]
                )
        nc.sync.dma_start(out=wq_sb0, in_=wq0_d[:, :, :])
        nc.sync.dma_start(out=xts4[1][0], in_=xt_d[1, 0])
        nc.sync.dma_start(out=xts4[1][1], in_=xt_d[1, 1])
        nc.sync.dma_start(out=wv_sb, in_=wv_d[:, :, :])
        nc.sync.dma_start(out=xts4[1][2], in_=xt_d[1, 2])
        nc.sync.dma_start(out=xts4[1][3], in_=xt_d[1, 3])
        dma_xt(2)
        nc.sync.dma_start(out=wq_sb1, in_=wq1_d[:, :, :])
        dma_xt(3)
        nc.sync.dma_start(out=wo_sb, in_=wo_d[:, :, :])
        k_half(0, 0, ppo, "ppo")
        k_half(0, 1, ppo, "ppo")
        q_half(0, 0, ppq, "ppq")
        q_half(0, 1, ppq, "ppq")
        v_chunk(0, 0, ppo, "ppo")
        v_chunk(0, 1, ppq, "ppq")
        v_chunk(0, 2, ppo, "ppo")
        v_chunk(0, 3, ppq, "ppq")

        # ---- filler schedule ----
        # round 0: remaining K/V projections just-in-time; later rounds:
        # O-proj chains of block j-1 plus the next Q chain
        sched = {}

        def add(it, kc, fn):
            sched.setdefault((it, kc), []).append(fn)

        add(0, 0, lambda: k_half(1, 0, ppo, "ppo"))
        add(0, 1, lambda: k_half(1, 1, ppo, "ppo"))
        add(0, 2, lambda: v_chunk(1, 0, ppq, "ppq"))
        add(0, 3, lambda: v_chunk(1, 1, ppq, "ppq"))
        add(0, 4, lambda: k_half(2, 0, ppo, "ppo"))
        add(0, 5, lambda: k_half(2, 1, ppo, "ppo"))
        add(0, 6, lambda: v_chunk(1, 2, ppq, "ppq"))
        add(0, 7, lambda: v_chunk(1, 3, ppq, "ppq"))
        add(0, 8, lambda: k_half(3, 0, ppo, "ppo"))
        add(0, 9, lambda: k_half(3, 1, ppo, "ppo"))
        add(0, 10, lambda: v_chunk(2, 0, ppq, "ppq"))
        add(0, 10, lambda: v_chunk(2, 1, ppq, "ppq"))
        add(0, 11, lambda: v_chunk(2, 2, ppq, "ppq"))
        add(0, 11, lambda: v_chunk(2, 3, ppq, "ppq"))
        add(0, 12, lambda: q_half(1, 0, ppo, "ppo"))
        add(0, 13, lambda: q_half(1, 1, ppo, "ppo"))
        add(0, 14, lambda: v_chunk(3, 0, ppq, "ppq"))
        add(0, 14, lambda: v_chunk(3, 1, ppq, "ppq"))
        add(0, 15, lambda: v_chunk(3, 2, ppq, "ppq"))
        add(0, 15, lambda: v_chunk(3, 3, ppq, "ppq"))
        def pe_fill(k):
            for _ in range(k):
                ps_j = ppq.tile([128, 512], F32, tag="ppq", name="junk")
                nc.tensor.matmul(out=ps_j, lhsT=e_sb, rhs=rl,
                                 start=True, stop=True)

        for it in range(1, ITERS):
            qi = it + 1
            if qi < ITERS:
                add(it, 5, lambda qi=qi: q_half(qi, 0, ppq, "ppq"))
                add(it, 6, lambda qi=qi: q_half(qi, 1, ppq, "ppq"))

        for it in range(1, ITERS):
            j, c = divmod(it, 4)
            if j > 0:
                slots = {10: 1, 12: 2, 14: 3, (15 if c == 0 else 1): 0}
                for kc, n in slots.items():
                    add(it, kc, lambda j=j, c=c, n=n: oproj_chain(j - 1, c, n))
            add(it, 4, lambda it=it: norm_recip(it - 1))
            add(it, 5, lambda it=it: norm_mul(it - 1, 0))
            add(it, 6, lambda it=it: norm_mul(it - 1, 1))

        # ---- flat global-slot pipeline, superslots of 2 chunks ----
        # S pairs for two consecutive chunks are emitted back-to-back so the
        # second pair's LDWEIGHTS hides under the first pair's streams, and
        # only one full matmul per superslot pays the post-tiled-pair LD tax
        sps_state = {}
        for m in range(ITERS * SC // 2):
            n0, n1 = 2 * m, 2 * m + 1
            it0, kc0 = divmod(n0, SC)
            it1, kc1 = divmod(n1, SC)
            emit_S_half(it0, kc0, 0)
            emit_S_half(it0, kc0, 1)
            emit_S_half(it1, kc1, 0)
            emit_S_half(it1, kc1, 1)
            if n0 >= 4:
                emit_PV_half(*divmod(n0 - 4, SC), 0)
                emit_PV_half(*divmod(n0 - 4, SC), 1)
            for fn in sched.get((it0, kc0), ()):
                fn()
            if n1 >= 4:
                emit_PV_half(*divmod(n1 - 4, SC), 0)
                emit_PV_half(*divmod(n1 - 4, SC), 1)
            for fn in sched.get((it1, kc1), ()):
                fn()
        # keep the PE (and HAM) busy while the tail exps drain on ScalarE;
        # junk matmuls have no deps so they run during the exp waits —
        # interleaved between tail PV pairs to bridge each exp's latency
        for n in range(ITERS * SC - 4, ITERS * SC):
            if n >= ITERS * SC - 2:
                pe_fill(3)
            emit_PV_half(*divmod(n, SC), 0)
            emit_PV_half(*divmod(n, SC), 1)
        itL = ITERS - 1
        jL, cL = divmod(itL, 4)
        ps_oA, ps_oB = pso_t[itL]
        nc.vector.reciprocal_approx_fast(out=rli, in_=psr_t[itL])
        # chunk the final normalize per 128-seq block so the drain's first
        # m-chain can start ~1.5us earlier
        for mm in range(4):
            s = slice(jL * 512 + mm * 128, jL * 512 + (mm + 1) * 128)
            r = slice(mm * 128, (mm + 1) * 128)
            nc.vector.tensor_mul(outT[cL][0:64, s], ps_oA[0:64, r], rli[0:64, r])
            nc.vector.tensor_mul(outT[cL][64:128, s], ps_oB[64:128, r], rli[64:128, r])

    # drain: O-projection of the last j-block via wide psum tiles; the
    # iteration pools above are closed, freeing PSUM and SBUF for wide
    # staging; copies split across DVE and ScalarE
    with tc.tile_pool(name="dps", bufs=2, space="PSUM") as dps, \
         tc.tile_pool(name="dst", bufs=3) as dst:
        for m in range(4):
            ms = slice(((NJ - 1) * 4 + m) * 128, ((NJ - 1) * 4 + m + 1) * 128)
            ps_w = dps.tile([128, HIDDEN], F32, tag="dw")
            st = dst.tile([128, HIDDEN], BF16, tag="da")
            for n in range(4):
                ns = slice(n * 512, (n + 1) * 512)
                for cc in range(4):
                    nc.tensor.matmul(
                        out=ps_w[:, ns],
                        lhsT=outT[cc][:, ms],
                        rhs=wo_sb[:, cc, ns],
                        start=(cc == 0), stop=(cc == 3),
                    )
                if n % 2 == 0:
                    nc.vector.tensor_copy(out=st[:, ns], in_=ps_w[:, ns])
                    nc.scalar.dma_start(out=out[ms, ns], in_=st[:, ns])
                else:
                    nc.scalar.copy(out=st[:, ns], in_=ps_w[:, ns])
                    nc.sync.dma_start(out=out[ms, ns], in_=st[:, ns])

    acts.release()
    consts.release()


_NC_CACHE = None


def _build():
    global _NC_CACHE
    if _NC_CACHE is None:
        nc = bacc.Bacc(
            "TRN2",
            target_bir_lowering=False,
            debug=False,
            enable_asserts=False,
            num_devices=N_CORES,
        )
        with tile.TileContext(nc) as tc:
            _body(tc)
        nc.compile()
        _NC_CACHE = nc
    return _NC_CACHE


def kernel(x, Wq, Wk, Wv, Wo):
    global LAST_EXEC_NS, LAST_RESULTS
    x = np.asarray(x, dtype=np.float32)
    Wq = np.asarray(Wq, dtype=np.float32)
    Wk = np.asarray(Wk, dtype=np.float32)
    Wv = np.asarray(Wv, dtype=np.float32)
    Wo = np.asarray(Wo, dtype=np.float32)
    bf = ml_dtypes.bfloat16

    in_maps = []
    for core in range(N_CORES):
        b, g = divmod(core, TP)
        qs = slice(g * QH * HEAD_DIM, (g + 1) * QH * HEAD_DIM)
        kvs = slice(g * KVH * HEAD_DIM, (g + 1) * KVH * HEAD_DIM)
        # (j, q, p, h, s): x[b][j*512+s, (q*4+h)*128+p] — quarter-major so
        # each quarter DMA is one contiguous 512KB DRAM run
        xtp = np.ascontiguousarray(
            x[b].reshape(NJ, 512, 4, 4, 128).transpose(0, 2, 4, 3, 1)
        ).astype(bf)
        wqp = np.ascontiguousarray(
            Wq[:, qs].reshape(HC, 128, 512).transpose(1, 0, 2)
        ).astype(bf)
        wq0 = np.ascontiguousarray(wqp[:, :, 0:128])
        wq1 = np.ascontiguousarray(wqp[:, :, 128:512])
        # (c, p, h, m): chunk-major so each of the 4 wk DMAs is contiguous
        wkp = np.ascontiguousarray(
            Wk[:, kvs].reshape(4, 4, 128, 128).transpose(0, 2, 1, 3)
        ).astype(bf)
        wvp = np.ascontiguousarray(
            Wv[:, kvs].reshape(HC, 128, 128).transpose(1, 0, 2)
        ).astype(bf)
        wop = np.ascontiguousarray(
            Wo[qs, :].reshape(4, 128, HIDDEN).transpose(1, 0, 2)
        ).astype(bf)
        in_maps.append({
            "xtp": xtp, "wq0": wq0, "wq1": wq1, "wkp": wkp, "wvp": wvp,
            "wop": wop,
        })

    nc = _build()
    res = run_bass_kernel_spmd(
        nc,
        in_maps,
        core_ids=list(range(N_CORES)),
        trace=PROFILE,
        trace_cores=list(range(N_CORES)) if PROFILE else None,
    )
    LAST_EXEC_NS = res.exec_time_ns
    LAST_RESULTS = res
    partials = [np.asarray(r["out"], dtype=np.float32) for r in res.results]
    out = np.empty((BATCH, SEQ, HIDDEN), dtype=np.float32)
    for b in range(BATCH):
        out[b] = partials[TP * b]
        for g in range(1, TP):
            out[b] += partials[TP * b + g]
    return out

